# revision 7
# baseline (speedup 1.0000x reference)
"""FBCritic embedding-lookup kernel for 8 Trainium2 NeuronCores.

Math (reference):
    fwd_idx = clip(obs)*10 + clip(act)            # [8192]
    bwd_idx = clip(fobs)*10 + clip(fact)          # [8192]
    F = W_f[fwd_idx]                              # [8192, 64]
    B = W_b[bwd_idx]                              # [8192, 64]
    out = F @ B.T                                 # [8192, 8192] f32

Sharding: data-parallel over the forward batch. Core c computes output rows
[c*1024, (c+1)*1024). Each core gathers its own 1024 forward rows and all
8192 backward rows from the (replicated) tables with indirect DMA. The HW
consumes exactly one index per destination partition, so a gather moves 128
table rows ([128, 64] tile); all 72 gathers are issued up front into a
72-buffer pool so the SWDGE queue streams them back-to-back (~1.04us each)
and transposes never wait on a late gather.

Pipeline: forward gathers -> 8 PE transposes -> fwdT [64, 1024] f32r. Then
per 1024-wide output column strip jp: 8 PE transposes + 2 PSUM->SBUF copies
build bt[jp] [64, 1024] f32r, and 8 row tiles each run 2 matmuls (N=512,
f32r = 1 cycle/row) into a 2-bank PSUM tile, one [128, 1024] PSUM->SBUF
copy (alternating scalar/vector), and one HWDGE DMA to HBM on the sync
queue. Keeping the PE stream dense lets it ramp to the 2.4 GHz p-state.
"""

import numpy as np

NUM_OBS = 100000
NUM_ACT = 10
V = NUM_OBS * NUM_ACT  # 1_000_000 table rows
D = 64                 # repr dim
B = 8192               # batch
N_CORES = 8
M = B // N_CORES       # 1024 output rows per core
P = 128                # partitions

_CACHE = {}


def _build_nc():
    import concourse.bass as bass
    import concourse.tile as tile
    from concourse import bacc, mybir
    from concourse.masks import make_identity

    f32 = mybir.dt.float32
    f32r = mybir.dt.float32r
    i32 = mybir.dt.int32

    nc = bacc.Bacc("TRN2", target_bir_lowering=False, debug=False)

    wf = nc.dram_tensor("wf", [V, D], f32, kind="ExternalInput").ap()
    wb = nc.dram_tensor("wb", [V, D], f32, kind="ExternalInput").ap()
    GF = M // P     # 8 forward 128-row groups
    GB = B // P     # 64 backward 128-row groups
    idx_d = nc.dram_tensor("idx", [P, GF + GB], i32, kind="ExternalInput").ap()
    out_d = nc.dram_tensor("out", [M, B], f32, kind="ExternalOutput").ap()

    NJ = 512        # matmul moving free dim (one PSUM bank)
    JP = 1024       # output strip width
    NPAIR = B // JP # 8 column strips

    n_copy = [0]

    def strip_copy(dst, src):
        if n_copy[0] % 2 == 0:
            nc.scalar.copy(out=dst, in_=src)
        else:
            nc.vector.tensor_copy(out=dst, in_=src)
        n_copy[0] += 1

    def gather128(pool, table, idx_tile, g):
        t = pool.tile([P, D], f32, tag="g")
        nc.gpsimd.indirect_dma_start(
            out=t[:],
            out_offset=None,
            in_=table[:],
            in_offset=bass.IndirectOffsetOnAxis(ap=idx_tile[:, g:g + 1], axis=0),
        )
        return t

    with tile.TileContext(nc) as tc:
        with (
            tc.tile_pool(name="const", bufs=1) as const_pool,
            tc.tile_pool(name="idx", bufs=1) as idx_pool,
            tc.tile_pool(name="g", bufs=GF + GB) as g_pool,
            tc.tile_pool(name="ops", bufs=1) as ops_pool,
            tc.tile_pool(name="strip", bufs=10) as strip_pool,
            tc.tile_pool(name="tpsum", bufs=2, space="PSUM") as tpsum_pool,
            tc.tile_pool(name="mpsum", bufs=3, space="PSUM") as mpsum_pool,
        ):
            identity = const_pool.tile([P, P], f32)
            make_identity(nc, identity[:])

            idxs = idx_pool.tile([P, GF + GB], i32, tag="idxs")
            nc.sync.dma_start(idxs[:], idx_d[:])
            idxf = idxs[:, 0:GF]
            idxb = idxs[:, GF:GF + GB]

            # Issue every gather up front; the SWDGE queue streams them
            # back-to-back while PE/vector/scalar consume earlier tiles.
            fgs = [gather128(g_pool, wf, idxf, g) for g in range(GF)]
            bgs = [gather128(g_pool, wb, idxb, g) for g in range(GB)]

            # Forward operand: 8 transposes -> fwdT [64, 1024] f32r.
            fwdT = ops_pool.tile([D, M], f32r, tag="fwdT")
            for q in range(GF // 4):
                pt = tpsum_pool.tile([D, 512], f32, tag="pt")
                for r in range(4):
                    nc.tensor.transpose(
                        out=pt[:, r * P:(r + 1) * P],
                        in_=fgs[q * 4 + r][:],
                        identity=identity[:],
                    )
                nc.vector.tensor_copy(out=fwdT[:, q * 512:(q + 1) * 512], in_=pt[:])

            # Column-strip-outer pipeline over the backward reprs.
            for jp in range(NPAIR):
                bt = ops_pool.tile([D, JP], f32r, tag=f"bwdT{jp}")
                for h in range(2):
                    pt = tpsum_pool.tile([D, 512], f32, tag="pt")
                    for r in range(4):
                        nc.tensor.transpose(
                            out=pt[:, r * P:(r + 1) * P],
                            in_=bgs[jp * 8 + h * 4 + r][:],
                            identity=identity[:],
                        )
                    nc.vector.tensor_copy(
                        out=bt[:, h * 512:(h + 1) * 512], in_=pt[:]
                    )

                for i in range(M // P):  # 8 row tiles
                    strip = strip_pool.tile([P, JP], f32, tag="strip")
                    ps = mpsum_pool.tile([P, JP], f32, tag="ps")  # 2 banks
                    for h in range(2):
                        nc.tensor.matmul(
                            out=ps[:, h * NJ:(h + 1) * NJ],
                            lhsT=fwdT[:, i * P:(i + 1) * P],
                            rhs=bt[:, h * NJ:(h + 1) * NJ],
                            start=True,
                            stop=True,
                        )
                    strip_copy(strip[:], ps[:])  # one [128, 1024] copy
                    nc.sync.dma_start(
                        out_d[i * P:(i + 1) * P, jp * JP:(jp + 1) * JP], strip[:]
                    )

    nc.compile()
    return nc


def _get_nc():
    if "nc" not in _CACHE:
        _CACHE["nc"] = _build_nc()
    return _CACHE["nc"]


def _ravel_clip(obs, act):
    o = np.clip(obs.astype(np.int64), 0, NUM_OBS - 1)
    a = np.clip(act.astype(np.int64), 0, NUM_ACT - 1)
    return (o * NUM_ACT + a).astype(np.int32)


def make_in_maps(observations, actions, future_observations, future_actions,
                 W_f, W_b):
    fwd_idx = _ravel_clip(np.asarray(observations), np.asarray(actions))
    bwd_idx = _ravel_clip(np.asarray(future_observations),
                          np.asarray(future_actions))
    wf = np.ascontiguousarray(np.asarray(W_f, dtype=np.float32))
    wb = np.ascontiguousarray(np.asarray(W_b, dtype=np.float32))
    # [p, g] = idx[g*128 + p]
    idxb = bwd_idx.reshape(B // P, P).T
    in_maps = []
    for c in range(N_CORES):
        idxf = fwd_idx[c * M:(c + 1) * M].reshape(M // P, P).T
        idx_all = np.ascontiguousarray(np.concatenate([idxf, idxb], axis=1))
        in_maps.append({"wf": wf, "wb": wb, "idx": idx_all})
    return in_maps


def kernel(**inputs):
    from concourse.bass_utils import run_bass_kernel_spmd

    in_maps = make_in_maps(
        inputs["observations"], inputs["actions"],
        inputs["future_observations"], inputs["future_actions"],
        inputs["W_f"], inputs["W_b"],
    )
    res = run_bass_kernel_spmd(_get_nc(), in_maps, core_ids=list(range(N_CORES)))
    return np.concatenate(
        [res.results[c]["out"] for c in range(N_CORES)], axis=0
    )
